# revision 16
# baseline (speedup 1.0000x reference)
"""Trainium2 Bass kernel for CoherenceNet masked-attention block (v3).

Math (per batch b, candidate half):
  scores[n, c] = sum_h attendeeT[h, n] * APT[h, c],   APT = W^T @ attenderT
  P = exp(scores - 100)          (global shift; softmax-invariant)
  PM = P * keep                  (keep = ~mask)
  d[c] = sum_n PM[n, c]          (masked denominator; ap_sz=1 matmuls)
  ctxT[h, c] = sum_n attendee[n, h] * PM[n, c]   (accumulated directly in
               transposed orientation -> no PE transposes anywhere)
  out[c, :] = tanh(attender[c] @ W1 + (ctxT_s[:,c]/d_s[c]) @ W2
                   + (ctxT_e[:,c]/d_e[c]) @ W3 + b_lin)
  1/d is applied per-partition (c) to the final-projection PSUM partials.

Scheduling notes:
  - HWDGE descriptor generation is one serial device (~630ns/DMA): all bulk
    loads are batched via rearranged access patterns, masks 8 n-tiles/DMA.
  - Software pipelining: ctx consumes PM at lag 3, denominator matmuls at
    lag 10 (so the d PSUM banks, shared with the final-projection partials
    of the previous chunk, are free in time).
  - Final projection for chunk cc is interleaved into chunk cc+1's stmt
    loop; normalization scaling runs on Act (Copy activation with
    per-partition scale AP), the adds on DVE.

Sharding: 8 cores = (batch b = core//2) x (candidate half = core%2).
"""

import numpy as np
import ml_dtypes

import concourse.bacc as bacc
import concourse.mybir as mybir
import concourse.tile as tile
from concourse.bass_utils import run_bass_kernel_spmd

B, S, E, C, H, A = 4, 4096, 2048, 4096, 256, 256
NCORES = 8
CL = C // 2
CHUNK = 512
NCHUNK = CL // CHUNK
SHIFT = -100.0
LAG = 3     # scores -> ctx pipeline distance (n-tiles)
LAGD = 10   # scores -> denominator pipeline distance (n-tiles)
KB = 8      # mask n-tiles per DMA

f32 = mybir.dt.float32
f32r = mybir.dt.float32r
bf16 = mybir.dt.bfloat16

_cache = {}


def _build():
    nc = bacc.Bacc("TRN2", target_bir_lowering=False, debug=False)

    atsT_d = nc.declare_dram_parameter("atsT", [H, S], f32r, isOutput=False)
    ateT_d = nc.declare_dram_parameter("ateT", [H, E], f32r, isOutput=False)
    atsn_d = nc.declare_dram_parameter("atsn", [S, H], bf16, isOutput=False)
    aten_d = nc.declare_dram_parameter("aten", [E, H], bf16, isOutput=False)
    atrT_d = nc.declare_dram_parameter("atrT", [H, CL], f32r, isOutput=False)
    atrTb_d = nc.declare_dram_parameter("atrTb", [H, CL], bf16, isOutput=False)
    wss_d = nc.declare_dram_parameter("wss", [H, H], f32r, isOutput=False)
    wes_d = nc.declare_dram_parameter("wes", [H, H], f32r, isOutput=False)
    wlinT_d = nc.declare_dram_parameter("wlinT", [3 * H, A], bf16, isOutput=False)
    blin_d = nc.declare_dram_parameter("blin", [1, A], bf16, isOutput=False)
    keeps_d = nc.declare_dram_parameter("keeps", [S, CL], bf16, isOutput=False)
    keepe_d = nc.declare_dram_parameter("keepe", [E, CL], bf16, isOutput=False)
    out_d = nc.declare_dram_parameter("out", [CL, A], f32, isOutput=True)

    NTS = S // 128   # 32 stmt n-tiles
    NTE = E // 128   # 16 ere n-tiles
    NBLK = CL // 128  # 16 final projection c-blocks

    keeps_r = keeps_d.rearrange("(i p) c -> p i c", p=128)
    keepe_r = keepe_d.rearrange("(i p) c -> p i c", p=128)
    atsn_r = atsn_d.rearrange("(i p) h -> p i h", p=128)
    aten_r = aten_d.rearrange("(i p) h -> p i h", p=128)
    wlin_r = wlinT_d.rearrange("(k p) a -> p k a", p=128)

    with tile.TileContext(nc) as tc:
        with (
            tc.tile_pool(name="res", bufs=1) as res,
            tc.tile_pool(name="pk", bufs=1) as pkp,
            tc.tile_pool(name="fin", bufs=2) as finp,
            tc.tile_pool(name="ps", bufs=1, space="PSUM") as psp,
        ):
            # ---------------- phase 0: constants + resident loads ----------
            # One serial DMA bus: order strictly by first use. First scores
            # needs wss + atrf[:, :, :1024] (APT cc0) + atsT sp0 only.
            wss_sb = res.tile([128, 2, H], f32r)
            wes_sb = res.tile([128, 2, H], f32r)
            atrf_sb = res.tile([128, 2, CL], f32r)
            atsT_sb = res.tile([128, 2, S], f32r)
            atsn_sb = res.tile([128, NTS, H], bf16)
            ateT_sb = res.tile([128, 2, E], f32r)
            aten_sb = res.tile([128, NTE, H], bf16)
            atrb_sb = res.tile([128, 2, CL], bf16)
            wlin_sb = res.tile([128, 6, A], bf16)
            blin_sb = res.tile([1, A], bf16)
            NSPL = 4
            spl = S // NSPL
            g = NTS // NSPL

            # Global mask-batch prefetch: batch list in consumption order;
            # each issued two batch-periods ahead of use (first two during
            # phase 0 on SP) so the multiply never waits on mask DMA.
            segs = []
            for kind in range(2):
                for cc in range(NCHUNK):
                    segs.append((cc, kind))
            gbatches = []
            for s_, (cc_, kind_) in enumerate(segs):
                nb = (NTS if kind_ == 0 else NTE) // KB
                for g_ in range(nb):
                    gbatches.append((s_, g_))
            gb_base = {}
            for gi_, (s_, g_) in enumerate(gbatches):
                if g_ == 0:
                    gb_base[s_] = gi_
            k_tiles = {}

            def issue_k(gi, eng=None):
                if gi >= len(gbatches):
                    return
                s, g = gbatches[gi]
                cc, kind = segs[s]
                keep_r = keeps_r if kind == 0 else keepe_r
                k_t = pkp.tile([128, KB, CHUNK], bf16, tag="K", bufs=3,
                               name="k_t")
                (eng or nc.scalar).dma_start(
                    out=k_t,
                    in_=keep_r[:, g * KB:(g + 1) * KB,
                               cc * CHUNK:(cc + 1) * CHUNK],
                )
                k_tiles[(s, g)] = k_t

            def load_ats(sp):
                for j in range(2):
                    nc.sync.dma_start(
                        out=atsT_sb[:, j, sp * spl:(sp + 1) * spl],
                        in_=atsT_d[j * 128:(j + 1) * 128,
                                   sp * spl:(sp + 1) * spl],
                    )
                nc.sync.dma_start(
                    out=atsn_sb[:, sp * g:(sp + 1) * g, :],
                    in_=atsn_r[:, sp * g:(sp + 1) * g, :],
                )

            nc.sync.dma_start(out=wss_sb[:, 0, :], in_=wss_d[0:128, :])
            nc.sync.dma_start(out=atrf_sb[:, 0, 0:1024],
                              in_=atrT_d[0:128, 0:1024])
            nc.sync.dma_start(out=wss_sb[:, 1, :], in_=wss_d[128:256, :])
            nc.sync.dma_start(out=atrf_sb[:, 1, 0:1024],
                              in_=atrT_d[128:256, 0:1024])
            load_ats(0)
            issue_k(0, nc.sync)
            issue_k(1, nc.sync)
            load_ats(1)
            load_ats(2)
            load_ats(3)

            # es-side inputs at the end of the SP stream: the bus serves
            # them after all ss-critical loads, before es segments need them
            for j in range(2):
                sl = slice(j * 128, (j + 1) * 128)
                nc.sync.dma_start(out=atrf_sb[:, j, 1024:CL],
                                  in_=atrT_d[sl, 1024:CL])
            for j in range(2):
                sl = slice(j * 128, (j + 1) * 128)
                nc.sync.dma_start(out=wes_sb[:, j, :], in_=wes_d[sl, :])
            for j in range(2):
                sl = slice(j * 128, (j + 1) * 128)
                nc.sync.dma_start(out=ateT_sb[:, j, :], in_=ateT_d[sl, :])
            nc.sync.dma_start(out=aten_sb[:, :, :], in_=aten_r[:, :, :])
            for j in range(2):
                nc.sync.dma_start(
                    out=atrb_sb[:, j, :],
                    in_=atrTb_d[j * 128:(j + 1) * 128, :],
                )
            nc.sync.dma_start(out=wlin_sb[:, :, :], in_=wlin_r[:, :, :])
            nc.sync.dma_start(out=blin_sb, in_=blin_d[:, :])

            onescol = res.tile([128, 1], bf16)
            nc.vector.memset(onescol, 1.0)
            onesrow = res.tile([1, 128], bf16)
            nc.vector.memset(onesrow, 1.0)
            negshift = res.tile([128, 1], f32)
            nc.vector.memset(negshift, SHIFT)

            # APT_X[h, c] = sum_h' W_X[h', h] attenderT[h', c]  (= W^T @ atrT)
            # Emitted per-chunk at segment start: fills PE at boundaries and
            # avoids blocking on late atrf halves.
            apt_ss = res.tile([128, 2, CL], f32r)
            apt_es = res.tile([128, 2, CL], f32r)

            def emit_apt(cc, kind):
                w_sb = wss_sb if kind == 0 else wes_sb
                apt = apt_ss if kind == 0 else apt_es
                if True:
                    for jj in range(2):      # output h-tile
                        pm = psp.tile([128, CHUNK], f32, tag=f"d{jj}", bufs=1)
                        for j in range(2):   # contraction tile
                            nc.tensor.matmul(
                                pm,
                                w_sb[:, j, jj * 128:(jj + 1) * 128],
                                atrf_sb[:, j, cc * CHUNK:(cc + 1) * CHUNK],
                                start=(j == 0),
                                stop=(j == 1),
                            )
                        nc.vector.tensor_copy(
                            apt[:, jj, cc * CHUNK:(cc + 1) * CHUNK], pm
                        )

            # ---------------- phase 1 + interleaved finals ------------------
            ctxT_s = res.tile([128, 2, CL], bf16)
            ctxT_e = res.tile([128, 2, CL], bf16)
            inv_s = res.tile([128, NBLK], f32)
            inv_e = res.tile([128, NBLK], f32)

            def emit_final(blk, tags=None, tail=False):
                qc = (blk % 4) * 128 + (blk // 4) * CHUNK
                t_att, t_cs, t_ce = tags or ((f"d{2 + blk % 2}", 1),
                                             ("d0", 1), ("d1", 1))
                pa_att = psp.tile([128, A], f32, tag=t_att[0], bufs=t_att[1],
                                  name="pa_att")
                nc.tensor.matmul(pa_att, onesrow, blin_sb[:, :],
                                 start=True, stop=False)
                pa_cs = psp.tile([128, A], f32, tag=t_cs[0], bufs=t_cs[1],
                                 name="pa_cs")
                pa_ce = psp.tile([128, A], f32, tag=t_ce[0], bufs=t_ce[1],
                                 name="pa_ce")
                for j in range(2):
                    nc.tensor.matmul(
                        pa_att, atrb_sb[:, j, qc:qc + 128], wlin_sb[:, j, :],
                        start=False, stop=(j == 1),
                    )
                    nc.tensor.matmul(
                        pa_cs, ctxT_s[:, j, qc:qc + 128], wlin_sb[:, 2 + j, :],
                        start=(j == 0), stop=(j == 1),
                    )
                    nc.tensor.matmul(
                        pa_ce, ctxT_e[:, j, qc:qc + 128], wlin_sb[:, 4 + j, :],
                        start=(j == 0), stop=(j == 1),
                    )
                blk16 = blk % NBLK
                t1 = finp.tile([128, A], f32, tag="t1")
                if tail:
                    nc.vector.tensor_scalar(
                        out=t1, in0=pa_cs, scalar1=inv_s[:, blk16:blk16 + 1],
                        scalar2=None, op0=mybir.AluOpType.mult,
                    )
                else:
                    nc.gpsimd.tensor_scalar(
                        out=t1, in0=pa_cs, scalar1=inv_s[:, blk16:blk16 + 1],
                        scalar2=None, op0=mybir.AluOpType.mult,
                    )
                t2 = finp.tile([128, A], f32, tag="t2")
                if tail:
                    nc.scalar.activation(
                        t2, pa_ce, mybir.ActivationFunctionType.Copy,
                        scale=inv_e[:, blk16:blk16 + 1],
                    )
                else:
                    nc.gpsimd.tensor_scalar(
                        out=t2, in0=pa_ce, scalar1=inv_e[:, blk16:blk16 + 1],
                        scalar2=None, op0=mybir.AluOpType.mult,
                    )
                t3 = finp.tile([128, A], f32, tag="t3")
                nc.vector.tensor_tensor(
                    out=t3, in0=pa_att, in1=t1, op=mybir.AluOpType.add
                )
                t4 = finp.tile([128, A], f32, tag="t4")
                nc.vector.tensor_tensor(
                    out=t4, in0=t3, in1=t2, op=mybir.AluOpType.add
                )
                ot = finp.tile([128, A], f32, tag="ot")
                nc.scalar.activation(
                    ot, t4, mybir.ActivationFunctionType.Tanh
                )
                nc.sync.dma_start(out=out_d[qc:qc + 128, :], in_=ot)

            for s, (cc, kind) in enumerate(segs):
                c0 = cc * CHUNK
                emit_apt(cc, kind)
                if True:
                    nts = NTS if kind == 0 else NTE
                    aT = atsT_sb if kind == 0 else ateT_sb
                    an = atsn_sb if kind == 0 else aten_sb
                    apt = apt_ss if kind == 0 else apt_es
                    ctxT = ctxT_s if kind == 0 else ctxT_e
                    inv = inv_s if kind == 0 else inv_e

                    ctx_ps = [
                        psp.tile([128, CHUNK], f32, tag=f"ctxh{hh}",
                                 name=f"ctx_ps{hh}")
                        for hh in range(2)
                    ]
                    d_ps = [
                        psp.tile([128, 1], f32, tag=f"d{q}", name=f"d_ps{q}")
                        for q in range(4)
                    ]
                    pm_tiles = {}
                    for it in range(nts + LAGD):
                        if it < nts and it % KB == 0:
                            issue_k(gb_base[s] + it // KB + 2)
                        if it < nts:
                            nt = it
                            sc = psp.tile([128, CHUNK], f32, tag="sc", bufs=2)
                            for j in range(2):
                                nc.tensor.matmul(
                                    sc,
                                    aT[:, j, nt * 128:(nt + 1) * 128],
                                    apt[:, j, c0:c0 + CHUNK],
                                    start=(j == 0),
                                    stop=(j == 1),
                                )
                            p_t = pkp.tile([128, CHUNK], bf16, tag="P",
                                           bufs=4)
                            nc.scalar.activation(
                                p_t, sc, mybir.ActivationFunctionType.Exp,
                                bias=negshift[:, :], scale=1.0,
                            )
                            pm_t = pkp.tile([128, CHUNK], bf16, tag="PM",
                                            bufs=LAGD + 2)
                            nc.vector.tensor_mul(
                                pm_t, p_t,
                                k_tiles[(s, nt // KB)][:, nt % KB, :]
                            )
                            pm_tiles[nt] = pm_t
                        # interleave previous chunk's final projections into
                        # the following ere segment
                        if kind == 1 and cc > 0 and it % 2 == 1 and it // 2 < 4:
                            emit_final((cc - 1) * 4 + it // 2)
                        if it >= LAG and it - LAG < nts:
                            nt = it - LAG
                            pm_t = pm_tiles[nt]
                            first = nt == 0
                            last = nt == nts - 1
                            for hh in range(2):
                                nc.tensor.matmul(
                                    ctx_ps[hh],
                                    an[:, nt, hh * 128:(hh + 1) * 128],
                                    pm_t,
                                    start=first,
                                    stop=last,
                                )
                        if it == nts - 1 + LAG:
                            for hh in range(2):
                                nc.gpsimd.tensor_copy(
                                    ctxT[:, hh, c0:c0 + CHUNK], ctx_ps[hh]
                                )
                        if it >= LAGD:
                            nt = it - LAGD
                            pm_t = pm_tiles.pop(nt)
                            first = nt == 0
                            last = nt == nts - 1
                            for q in range(4):
                                nc.tensor.matmul(
                                    d_ps[q],
                                    pm_t[:, q * 128:(q + 1) * 128],
                                    onescol,
                                    start=first,
                                    stop=last,
                                )

                    for q in range(4):
                        nc.vector.reciprocal(
                            inv[:, cc * 4 + q:cc * 4 + q + 1], d_ps[q]
                        )

            # last chunk's final projections (tail); sc/ctxh banks are free
            for blk in range((NCHUNK - 1) * 4, NCHUNK * 4):
                emit_final(blk, tags=(("sc", 2), ("sc", 2), ("ctxh0", 1)), tail=True)

    nc.compile()
    return nc


def _make_in_maps(attendee_stmts, attendee_eres, attender, W_ss, W_es,
                  W_lin, b_lin, mask_stmt_to_stmt, mask_ere_to_stmt):
    bfd = ml_dtypes.bfloat16
    attendee_stmts = np.asarray(attendee_stmts, dtype=np.float32)
    attendee_eres = np.asarray(attendee_eres, dtype=np.float32)
    attender = np.asarray(attender, dtype=np.float32)
    W_ss = np.ascontiguousarray(np.asarray(W_ss, dtype=np.float32))
    W_es = np.ascontiguousarray(np.asarray(W_es, dtype=np.float32))
    wlinT = np.ascontiguousarray(np.asarray(W_lin, dtype=np.float32).T
                                 .astype(bfd))
    blin = np.asarray(b_lin, dtype=np.float32).reshape(1, A).astype(bfd)
    keep_s = (~np.asarray(mask_stmt_to_stmt)).astype(bfd)
    keep_e = (~np.asarray(mask_ere_to_stmt)).astype(bfd)

    per_b = {}
    for b in range(B):
        per_b[b] = {
            "atsT": np.ascontiguousarray(attendee_stmts[b].T),
            "ateT": np.ascontiguousarray(attendee_eres[b].T),
            "atsn": np.ascontiguousarray(attendee_stmts[b].astype(bfd)),
            "aten": np.ascontiguousarray(attendee_eres[b].astype(bfd)),
        }

    in_maps = []
    for core in range(NCORES):
        b = core // 2
        h0 = (core % 2) * CL
        atrT = np.ascontiguousarray(attender[b, h0:h0 + CL].T)
        in_maps.append({
            **per_b[b],
            "atrT": atrT,
            "atrTb": np.ascontiguousarray(atrT.astype(bfd)),
            "wss": W_ss,
            "wes": W_es,
            "wlinT": wlinT,
            "blin": blin,
            "keeps": np.ascontiguousarray(keep_s[b, :, h0:h0 + CL]),
            "keepe": np.ascontiguousarray(keep_e[b, :, h0:h0 + CL]),
        })
    return in_maps


def kernel(attendee_stmts, attendee_eres, attender, W_ss, b_ss, W_es, b_es,
           W_lin, b_lin, mask_stmt_to_stmt, mask_ere_to_stmt):
    if "nc" not in _cache:
        _cache["nc"] = _build()
    nc = _cache["nc"]

    in_maps = _make_in_maps(attendee_stmts, attendee_eres, attender,
                            W_ss, W_es, W_lin, b_lin,
                            mask_stmt_to_stmt, mask_ere_to_stmt)

    res = run_bass_kernel_spmd(nc, in_maps, core_ids=list(range(NCORES)))

    out = np.empty((B, C, A), dtype=np.float32)
    for core in range(NCORES):
        b = core // 2
        h0 = (core % 2) * CL
        out[b, h0:h0 + CL] = res.results[core]["out"]
    return out


# revision 17
# speedup vs baseline: 1.0100x; 1.0100x over previous
"""Trainium2 Bass kernel for CoherenceNet masked-attention block (v3).

Math (per batch b, candidate half):
  scores[n, c] = sum_h attendeeT[h, n] * APT[h, c],   APT = W^T @ attenderT
  P = exp(scores - 100)          (global shift; softmax-invariant)
  PM = P * keep                  (keep = ~mask)
  d[c] = sum_n PM[n, c]          (masked denominator; ap_sz=1 matmuls)
  ctxT[h, c] = sum_n attendee[n, h] * PM[n, c]   (accumulated directly in
               transposed orientation -> no PE transposes anywhere)
  out[c, :] = tanh(attender[c] @ W1 + (ctxT_s[:,c]/d_s[c]) @ W2
                   + (ctxT_e[:,c]/d_e[c]) @ W3 + b_lin)
  1/d is applied per-partition (c) to the final-projection PSUM partials.

Scheduling notes:
  - HWDGE descriptor generation is one serial device (~630ns/DMA): all bulk
    loads are batched via rearranged access patterns, masks 8 n-tiles/DMA.
  - Software pipelining: ctx consumes PM at lag 3, denominator matmuls at
    lag 10 (so the d PSUM banks, shared with the final-projection partials
    of the previous chunk, are free in time).
  - Final projection for chunk cc is interleaved into chunk cc+1's stmt
    loop; normalization scaling runs on Act (Copy activation with
    per-partition scale AP), the adds on DVE.

Sharding: 8 cores = (batch b = core//2) x (candidate half = core%2).
"""

import numpy as np
import ml_dtypes

import concourse.bacc as bacc
import concourse.mybir as mybir
import concourse.tile as tile
from concourse.bass_utils import run_bass_kernel_spmd

B, S, E, C, H, A = 4, 4096, 2048, 4096, 256, 256
NCORES = 8
CL = C // 2
CHUNK = 512
NCHUNK = CL // CHUNK
SHIFT = -100.0
LAG = 3     # scores -> ctx pipeline distance (n-tiles)
LAGD = 10   # scores -> denominator pipeline distance (n-tiles)
KB = 8      # mask n-tiles per DMA

f32 = mybir.dt.float32
f32r = mybir.dt.float32r
bf16 = mybir.dt.bfloat16

_cache = {}


def _build():
    nc = bacc.Bacc("TRN2", target_bir_lowering=False, debug=False)

    atsT_d = nc.declare_dram_parameter("atsT", [H, S], f32r, isOutput=False)
    ateT_d = nc.declare_dram_parameter("ateT", [H, E], f32r, isOutput=False)
    atsn_d = nc.declare_dram_parameter("atsn", [S, H], bf16, isOutput=False)
    aten_d = nc.declare_dram_parameter("aten", [E, H], bf16, isOutput=False)
    atrT_d = nc.declare_dram_parameter("atrT", [H, CL], f32r, isOutput=False)
    atrTb_d = nc.declare_dram_parameter("atrTb", [H, CL], bf16, isOutput=False)
    wss_d = nc.declare_dram_parameter("wss", [H, H], f32r, isOutput=False)
    wes_d = nc.declare_dram_parameter("wes", [H, H], f32r, isOutput=False)
    wlinT_d = nc.declare_dram_parameter("wlinT", [3 * H, A], bf16, isOutput=False)
    blin_d = nc.declare_dram_parameter("blin", [1, A], bf16, isOutput=False)
    keeps_d = nc.declare_dram_parameter("keeps", [S, CL], bf16, isOutput=False)
    keepe_d = nc.declare_dram_parameter("keepe", [E, CL], bf16, isOutput=False)
    out_d = nc.declare_dram_parameter("out", [CL, A], f32, isOutput=True)

    NTS = S // 128   # 32 stmt n-tiles
    NTE = E // 128   # 16 ere n-tiles
    NBLK = CL // 128  # 16 final projection c-blocks

    keeps_r = keeps_d.rearrange("(i p) c -> p i c", p=128)
    keepe_r = keepe_d.rearrange("(i p) c -> p i c", p=128)
    atsn_r = atsn_d.rearrange("(i p) h -> p i h", p=128)
    aten_r = aten_d.rearrange("(i p) h -> p i h", p=128)
    wlin_r = wlinT_d.rearrange("(k p) a -> p k a", p=128)

    with tile.TileContext(nc) as tc:
        with (
            tc.tile_pool(name="res", bufs=1) as res,
            tc.tile_pool(name="pk", bufs=1) as pkp,
            tc.tile_pool(name="fin", bufs=2) as finp,
            tc.tile_pool(name="ps", bufs=1, space="PSUM") as psp,
        ):
            # ---------------- phase 0: constants + resident loads ----------
            # One serial DMA bus: order strictly by first use. First scores
            # needs wss + atrf[:, :, :1024] (APT cc0) + atsT sp0 only.
            wss_sb = res.tile([128, 2, H], f32r)
            wes_sb = res.tile([128, 2, H], f32r)
            atrf_sb = res.tile([128, 2, CL], f32r)
            atsT_sb = res.tile([128, 2, S], f32r)
            atsn_sb = res.tile([128, NTS, H], bf16)
            ateT_sb = res.tile([128, 2, E], f32r)
            aten_sb = res.tile([128, NTE, H], bf16)
            atrb_sb = res.tile([128, 2, CL], bf16)
            wlin_sb = res.tile([128, 6, A], bf16)
            blin_sb = res.tile([1, A], bf16)
            NSPL = 4
            spl = S // NSPL
            g = NTS // NSPL

            # Global mask-batch prefetch: batch list in consumption order;
            # each issued two batch-periods ahead of use (first two during
            # phase 0 on SP) so the multiply never waits on mask DMA.
            segs = []
            for kind in range(2):
                for cc in range(NCHUNK):
                    segs.append((cc, kind))
            gbatches = []
            for s_, (cc_, kind_) in enumerate(segs):
                nb = (NTS if kind_ == 0 else NTE) // KB
                for g_ in range(nb):
                    gbatches.append((s_, g_))
            gb_base = {}
            for gi_, (s_, g_) in enumerate(gbatches):
                if g_ == 0:
                    gb_base[s_] = gi_
            k_tiles = {}

            def issue_k(gi, eng=None):
                if gi >= len(gbatches):
                    return
                s, g = gbatches[gi]
                cc, kind = segs[s]
                keep_r = keeps_r if kind == 0 else keepe_r
                k_t = pkp.tile([128, KB, CHUNK], bf16, tag="K", bufs=3,
                               name="k_t")
                (eng or nc.scalar).dma_start(
                    out=k_t,
                    in_=keep_r[:, g * KB:(g + 1) * KB,
                               cc * CHUNK:(cc + 1) * CHUNK],
                )
                k_tiles[(s, g)] = k_t

            def load_ats(sp):
                for j in range(2):
                    nc.sync.dma_start(
                        out=atsT_sb[:, j, sp * spl:(sp + 1) * spl],
                        in_=atsT_d[j * 128:(j + 1) * 128,
                                   sp * spl:(sp + 1) * spl],
                    )
                nc.sync.dma_start(
                    out=atsn_sb[:, sp * g:(sp + 1) * g, :],
                    in_=atsn_r[:, sp * g:(sp + 1) * g, :],
                )

            nc.sync.dma_start(out=wss_sb[:, 0, :], in_=wss_d[0:128, :])
            nc.sync.dma_start(out=atrf_sb[:, 0, 0:1024],
                              in_=atrT_d[0:128, 0:1024])
            nc.sync.dma_start(out=wss_sb[:, 1, :], in_=wss_d[128:256, :])
            nc.sync.dma_start(out=atrf_sb[:, 1, 0:1024],
                              in_=atrT_d[128:256, 0:1024])
            load_ats(0)
            issue_k(0, nc.sync)
            issue_k(1, nc.sync)
            load_ats(1)
            load_ats(2)
            load_ats(3)

            # es-side inputs at the end of the SP stream: the bus serves
            # them after all ss-critical loads, before es segments need them
            for j in range(2):
                sl = slice(j * 128, (j + 1) * 128)
                nc.sync.dma_start(out=atrf_sb[:, j, 1024:CL],
                                  in_=atrT_d[sl, 1024:CL])
            for j in range(2):
                sl = slice(j * 128, (j + 1) * 128)
                nc.sync.dma_start(out=wes_sb[:, j, :], in_=wes_d[sl, :])
            for j in range(2):
                sl = slice(j * 128, (j + 1) * 128)
                nc.sync.dma_start(out=ateT_sb[:, j, :], in_=ateT_d[sl, :])
            nc.sync.dma_start(out=aten_sb[:, :, :], in_=aten_r[:, :, :])
            for j in range(2):
                nc.sync.dma_start(
                    out=atrb_sb[:, j, :],
                    in_=atrTb_d[j * 128:(j + 1) * 128, :],
                )
            nc.sync.dma_start(out=wlin_sb[:, :, :], in_=wlin_r[:, :, :])
            nc.sync.dma_start(out=blin_sb, in_=blin_d[:, :])

            onescol = res.tile([128, 1], bf16)
            nc.vector.memset(onescol, 1.0)
            onesrow = res.tile([1, 128], bf16)
            nc.vector.memset(onesrow, 1.0)
            negshift = res.tile([128, 1], f32)
            nc.vector.memset(negshift, SHIFT)

            # APT_X[h, c] = sum_h' W_X[h', h] attenderT[h', c]  (= W^T @ atrT)
            # Emitted per-chunk at segment start: fills PE at boundaries and
            # avoids blocking on late atrf halves.
            apt_ss = res.tile([128, 2, CL], f32r)
            apt_es = res.tile([128, 2, CL], f32r)

            def emit_apt(cc, kind):
                w_sb = wss_sb if kind == 0 else wes_sb
                apt = apt_ss if kind == 0 else apt_es
                if True:
                    for jj in range(2):      # output h-tile
                        pm = psp.tile([128, CHUNK], f32, tag="sc", bufs=2)
                        for j in range(2):   # contraction tile
                            nc.tensor.matmul(
                                pm,
                                w_sb[:, j, jj * 128:(jj + 1) * 128],
                                atrf_sb[:, j, cc * CHUNK:(cc + 1) * CHUNK],
                                start=(j == 0),
                                stop=(j == 1),
                            )
                        nc.vector.tensor_copy(
                            apt[:, jj, cc * CHUNK:(cc + 1) * CHUNK], pm
                        )

            # ---------------- phase 1 + interleaved finals ------------------
            ctxT_s = res.tile([128, 2, CL], bf16)
            ctxT_e = res.tile([128, 2, CL], bf16)
            inv_s = res.tile([128, NBLK], f32)
            inv_e = res.tile([128, NBLK], f32)

            def emit_final(blk, tags=None, tail=False):
                qc = (blk % 4) * 128 + (blk // 4) * CHUNK
                t_att, t_cs, t_ce = tags or ((f"d{2 + blk % 2}", 1),
                                             ("d0", 1), ("d1", 1))
                pa_att = psp.tile([128, A], f32, tag=t_att[0], bufs=t_att[1],
                                  name="pa_att")
                nc.tensor.matmul(pa_att, onesrow, blin_sb[:, :],
                                 start=True, stop=False)
                pa_cs = psp.tile([128, A], f32, tag=t_cs[0], bufs=t_cs[1],
                                 name="pa_cs")
                pa_ce = psp.tile([128, A], f32, tag=t_ce[0], bufs=t_ce[1],
                                 name="pa_ce")
                for j in range(2):
                    nc.tensor.matmul(
                        pa_att, atrb_sb[:, j, qc:qc + 128], wlin_sb[:, j, :],
                        start=False, stop=(j == 1),
                    )
                    nc.tensor.matmul(
                        pa_cs, ctxT_s[:, j, qc:qc + 128], wlin_sb[:, 2 + j, :],
                        start=(j == 0), stop=(j == 1),
                    )
                    nc.tensor.matmul(
                        pa_ce, ctxT_e[:, j, qc:qc + 128], wlin_sb[:, 4 + j, :],
                        start=(j == 0), stop=(j == 1),
                    )
                blk16 = blk % NBLK
                t1 = finp.tile([128, A], f32, tag="t1")
                if tail:
                    nc.vector.tensor_scalar(
                        out=t1, in0=pa_cs, scalar1=inv_s[:, blk16:blk16 + 1],
                        scalar2=None, op0=mybir.AluOpType.mult,
                    )
                else:
                    nc.gpsimd.tensor_scalar(
                        out=t1, in0=pa_cs, scalar1=inv_s[:, blk16:blk16 + 1],
                        scalar2=None, op0=mybir.AluOpType.mult,
                    )
                t2 = finp.tile([128, A], f32, tag="t2")
                if tail:
                    nc.scalar.activation(
                        t2, pa_ce, mybir.ActivationFunctionType.Copy,
                        scale=inv_e[:, blk16:blk16 + 1],
                    )
                else:
                    nc.gpsimd.tensor_scalar(
                        out=t2, in0=pa_ce, scalar1=inv_e[:, blk16:blk16 + 1],
                        scalar2=None, op0=mybir.AluOpType.mult,
                    )
                t3 = finp.tile([128, A], f32, tag="t3")
                nc.vector.tensor_tensor(
                    out=t3, in0=pa_att, in1=t1, op=mybir.AluOpType.add
                )
                t4 = finp.tile([128, A], f32, tag="t4")
                nc.vector.tensor_tensor(
                    out=t4, in0=t3, in1=t2, op=mybir.AluOpType.add
                )
                ot = finp.tile([128, A], f32, tag="ot")
                nc.scalar.activation(
                    ot, t4, mybir.ActivationFunctionType.Tanh
                )
                nc.sync.dma_start(out=out_d[qc:qc + 128, :], in_=ot)

            for s, (cc, kind) in enumerate(segs):
                c0 = cc * CHUNK
                emit_apt(cc, kind)
                if True:
                    nts = NTS if kind == 0 else NTE
                    aT = atsT_sb if kind == 0 else ateT_sb
                    an = atsn_sb if kind == 0 else aten_sb
                    apt = apt_ss if kind == 0 else apt_es
                    ctxT = ctxT_s if kind == 0 else ctxT_e
                    inv = inv_s if kind == 0 else inv_e

                    ctx_ps = [
                        psp.tile([128, CHUNK], f32, tag=f"ctxh{hh}",
                                 name=f"ctx_ps{hh}")
                        for hh in range(2)
                    ]
                    d_ps = [
                        psp.tile([128, 1], f32, tag=f"d{q}", name=f"d_ps{q}")
                        for q in range(4)
                    ]
                    pm_tiles = {}
                    for it in range(nts + LAGD):
                        if it < nts and it % KB == 0:
                            issue_k(gb_base[s] + it // KB + 2)
                        if it < nts:
                            nt = it
                            sc = psp.tile([128, CHUNK], f32, tag="sc", bufs=2)
                            for j in range(2):
                                nc.tensor.matmul(
                                    sc,
                                    aT[:, j, nt * 128:(nt + 1) * 128],
                                    apt[:, j, c0:c0 + CHUNK],
                                    start=(j == 0),
                                    stop=(j == 1),
                                )
                            p_t = pkp.tile([128, CHUNK], bf16, tag="P",
                                           bufs=4)
                            nc.scalar.activation(
                                p_t, sc, mybir.ActivationFunctionType.Exp,
                                bias=negshift[:, :], scale=1.0,
                            )
                            pm_t = pkp.tile([128, CHUNK], bf16, tag="PM",
                                            bufs=LAGD + 2)
                            nc.vector.tensor_mul(
                                pm_t, p_t,
                                k_tiles[(s, nt // KB)][:, nt % KB, :]
                            )
                            pm_tiles[nt] = pm_t
                        # interleave previous chunk's final projections into
                        # the following ere segment
                        if kind == 1 and cc > 0 and it % 2 == 1 and it // 2 < 4:
                            emit_final((cc - 1) * 4 + it // 2)
                        if it >= LAG and it - LAG < nts:
                            nt = it - LAG
                            pm_t = pm_tiles[nt]
                            first = nt == 0
                            last = nt == nts - 1
                            for hh in range(2):
                                nc.tensor.matmul(
                                    ctx_ps[hh],
                                    an[:, nt, hh * 128:(hh + 1) * 128],
                                    pm_t,
                                    start=first,
                                    stop=last,
                                )
                        if it == nts - 1 + LAG:
                            for hh in range(2):
                                nc.gpsimd.tensor_copy(
                                    ctxT[:, hh, c0:c0 + CHUNK], ctx_ps[hh]
                                )
                        if it >= LAGD:
                            nt = it - LAGD
                            pm_t = pm_tiles.pop(nt)
                            first = nt == 0
                            last = nt == nts - 1
                            for q in range(4):
                                nc.tensor.matmul(
                                    d_ps[q],
                                    pm_t[:, q * 128:(q + 1) * 128],
                                    onescol,
                                    start=first,
                                    stop=last,
                                )

                    for q in range(4):
                        nc.vector.reciprocal(
                            inv[:, cc * 4 + q:cc * 4 + q + 1], d_ps[q]
                        )

            # last chunk's final projections (tail); sc/ctxh banks are free
            for blk in range((NCHUNK - 1) * 4, NCHUNK * 4):
                emit_final(blk, tags=(("sc", 2), ("sc", 2), ("ctxh0", 1)), tail=True)

    nc.compile()
    return nc


def _make_in_maps(attendee_stmts, attendee_eres, attender, W_ss, W_es,
                  W_lin, b_lin, mask_stmt_to_stmt, mask_ere_to_stmt):
    bfd = ml_dtypes.bfloat16
    attendee_stmts = np.asarray(attendee_stmts, dtype=np.float32)
    attendee_eres = np.asarray(attendee_eres, dtype=np.float32)
    attender = np.asarray(attender, dtype=np.float32)
    W_ss = np.ascontiguousarray(np.asarray(W_ss, dtype=np.float32))
    W_es = np.ascontiguousarray(np.asarray(W_es, dtype=np.float32))
    wlinT = np.ascontiguousarray(np.asarray(W_lin, dtype=np.float32).T
                                 .astype(bfd))
    blin = np.asarray(b_lin, dtype=np.float32).reshape(1, A).astype(bfd)
    keep_s = (~np.asarray(mask_stmt_to_stmt)).astype(bfd)
    keep_e = (~np.asarray(mask_ere_to_stmt)).astype(bfd)

    per_b = {}
    for b in range(B):
        per_b[b] = {
            "atsT": np.ascontiguousarray(attendee_stmts[b].T),
            "ateT": np.ascontiguousarray(attendee_eres[b].T),
            "atsn": np.ascontiguousarray(attendee_stmts[b].astype(bfd)),
            "aten": np.ascontiguousarray(attendee_eres[b].astype(bfd)),
        }

    in_maps = []
    for core in range(NCORES):
        b = core // 2
        h0 = (core % 2) * CL
        atrT = np.ascontiguousarray(attender[b, h0:h0 + CL].T)
        in_maps.append({
            **per_b[b],
            "atrT": atrT,
            "atrTb": np.ascontiguousarray(atrT.astype(bfd)),
            "wss": W_ss,
            "wes": W_es,
            "wlinT": wlinT,
            "blin": blin,
            "keeps": np.ascontiguousarray(keep_s[b, :, h0:h0 + CL]),
            "keepe": np.ascontiguousarray(keep_e[b, :, h0:h0 + CL]),
        })
    return in_maps


def kernel(attendee_stmts, attendee_eres, attender, W_ss, b_ss, W_es, b_es,
           W_lin, b_lin, mask_stmt_to_stmt, mask_ere_to_stmt):
    if "nc" not in _cache:
        _cache["nc"] = _build()
    nc = _cache["nc"]

    in_maps = _make_in_maps(attendee_stmts, attendee_eres, attender,
                            W_ss, W_es, W_lin, b_lin,
                            mask_stmt_to_stmt, mask_ere_to_stmt)

    res = run_bass_kernel_spmd(nc, in_maps, core_ids=list(range(NCORES)))

    out = np.empty((B, C, A), dtype=np.float32)
    for core in range(NCORES):
        b = core // 2
        h0 = (core % 2) * CL
        out[b, h0:h0 + CL] = res.results[core]["out"]
    return out
